# revision 1
# baseline (speedup 1.0000x reference)
"""DSVDD retrieval-knn kernel for 8 Trainium2 NeuronCores.

Data-parallel over batch: one image per NeuronCore, no collectives. Per image:

  stage A  inputs are scaled 0.25x and cast bf16 (ACT), 3x3 sum-pooled
           (DVE tensor_add chains, all-bf16 for 2x mode; the /9 of
           avg-pooling and all scales fold into the weights), p2 is
           2x-bilinear-upsampled in difference form (d = x1-x0; each output
           phase is one scalar_tensor_tensor) -> xc chunks 0..5 live in a
           RESIDENT fp8 SBUF tile (no DRAM roundtrip).
           p3 is pooled only: psi = W3^T pool(p3) runs at native 14x14
           (TensorE bf16), and the 1792-dim psi maps are 4x-bilinear-
           upsampled (difference form, DVE+GPSIMD) -> phi3 fp8 in DRAM.
           Linearity of resize o 1x1-conv makes this exact and cuts the
           full-resolution contraction from 1794 to 770 channels.
  mm1      phi = W12^T xc + b + phi3 with W12/xc in fp8e4 (DoubleRow pairs,
           2 channel blocks per matmul); coords contribute via a K=2 bf16
           matmul; PSUM eviction (+bias +phi3, cast fp8) runs on GPSIMD.
  mm2      PSUM = SS*(2 phi.C - |C|^2 - |phi|^2) = -SS*dist^2 entirely in
           fp8e4 DoubleRow (phi scale 16, centroid scale 32 -> SS = 512);
           |phi|^2 rows come from ACT Square(0.25*phi) -> fp8 and a
           DoubleRow ones-matmul; the norm rows enter as a K=4 bf16 matmul
           (hi/lo bf16 splits keep them exact enough).
  top-3    vector.max (max8) straight on PSUM per 448-column block, then a
           merge max8; vals = sqrt(-top3/SS) (ACT); softmin -> w0*v0
           (batched over all 25 row-blocks at the end).

All scale factors are powers of two, so rescaling is exact; |C|^2 is
computed host-side from the fp8-quantized centroids so dist^2 stays a true
squared distance of quantized vectors.
"""
import os, sys

sys.path.insert(0, os.environ.get("TRN_RL_REPO", "/opt/trn_rl_repo"))

import numpy as np
import ml_dtypes

import concourse.bass as bass
import concourse.tile as tile
from concourse import bacc, mybir

F32 = mybir.dt.float32
BF16 = mybir.dt.bfloat16
F8 = mybir.dt.float8e4
AF = mybir.ActivationFunctionType
ALU = mybir.AluOpType
AX = mybir.AxisListType
DR = mybir.MatmulPerfMode.DoubleRow

N_CORES = 8
HW = 3136          # 56*56
NCEN = 3136        # centroids
CO = 1792          # output channels / feature dim
MCH = 14           # CO / 128
HW_CHUNKS = [(i * 512, 512) for i in range(6)] + [(3072, 64)]
# mm2 centroid chunks: uniform 448 wide so every fp8 DoubleRow matmul has
# free dim >= 256.
CEN_CHUNKS = [(i * 448, 448) for i in range(7)]
# Scale plan (exact powers of two):
#   inputs x 0.25 at the bf16 convert; fp8 W12 carries 64/9 -> phi PSUM and
#   the evicted fp8 phi are 16*phi.  Centroids are fp8(64*C) = 2*S_C*C.
#   mm2 PSUM = -SS*dist^2 with SS = S_PHI*S_C = 512, undone in the sqrt.
S_PHI = 16.0
S_C = 32.0
SS = S_PHI * S_C
S_W8 = 64.0        # weight scale into fp8 (inputs carry 1/4)
X_SCALE = 0.25


# ---------------------------------------------------------------- stage A ---
def _pool3(nc, pool, xv, H, W, out, idx, final_eng=None):
    """out[c,h,w] = sum of 3x3 zero-padded neighborhood of xv (no /9).

    xv: [128, H, W] bf16 AP (pre-scaled). out: [128, H, W] AP (any dtype).
    All intermediate adds are bf16-only so DVE runs them in 2x mode.
    """
    fe = final_eng or nc.vector
    a = pool.tile([128, H, W], BF16, name=f"pa_{idx}", tag="pa")
    nc.vector.tensor_add(a[:, :, 0:W - 1], xv[:, :, 0:W - 1], xv[:, :, 1:W])
    nc.scalar.copy(a[:, :, W - 1:W], xv[:, :, W - 1:W])
    s = pool.tile([128, H, W], BF16, name=f"ps_{idx}", tag="ps")
    nc.vector.tensor_add(s[:, :, 1:W], a[:, :, 1:W], xv[:, :, 0:W - 1])
    nc.scalar.copy(s[:, :, 0:1], a[:, :, 0:1])
    t = pool.tile([128, H, W], BF16, name=f"pt_{idx}", tag="pa")
    nc.vector.tensor_add(t[:, 0:H - 1, :], s[:, 0:H - 1, :], s[:, 1:H, :])
    nc.scalar.copy(t[:, H - 1:H, :], s[:, H - 1:H, :])
    fe.tensor_add(out[:, 1:H, :], t[:, 1:H, :], s[:, 0:H - 1, :])
    nc.scalar.copy(out[:, 0:1, :], t[:, 0:1, :])


def _up2_w(nc, pool, q, H, W, out, idx):
    """q [128,H,W] bf16 -> out [128,H,2W], half-pixel bilinear 2x on last
    axis, difference form: out[2a+1] = q[a] + d[a]/4, out[2a] = q[a] - d[a-1]/4.
    """
    d = pool.tile([128, H, W - 1], BF16, name=f"u2wd_{idx}", tag="u2d")
    nc.vector.tensor_sub(d[:], q[:, :, 1:W], q[:, :, 0:W - 1])
    ov = out.rearrange("p h (a two) -> p h a two", two=2)
    nc.vector.scalar_tensor_tensor(
        out=ov[:, :, 0:W - 1, 1:2], in0=d[:], scalar=0.25,
        in1=q[:, :, 0:W - 1], op0=ALU.mult, op1=ALU.add)
    nc.vector.scalar_tensor_tensor(
        out=ov[:, :, 1:W, 0:1], in0=d[:], scalar=-0.25,
        in1=q[:, :, 1:W], op0=ALU.mult, op1=ALU.add)
    nc.scalar.copy(ov[:, :, 0:1, 0:1], q[:, :, 0:1])
    nc.scalar.copy(ov[:, :, W - 1:W, 1:2], q[:, :, W - 1:W])


def _up2_h(nc, pool, rh, H, W2, out, idx):
    """rh [128,H,W2] bf16 -> out [128,2H,W2] (out may be fp8). The lane-1
    phase is decomposed (tsmul + add) and runs fully on GPSIMD (which has no
    scalar_tensor_tensor)."""
    d = pool.tile([128, H - 1, W2], BF16, name=f"u2hd_{idx}", tag="u2dh")
    nc.vector.tensor_sub(d[:], rh[:, 1:H, :], rh[:, 0:H - 1, :])
    ov = out.rearrange("p (a two) w -> p a two w", two=2)
    nc.vector.scalar_tensor_tensor(
        out=ov[:, 0:H - 1, 1:2, :], in0=d[:], scalar=0.25,
        in1=rh[:, 0:H - 1, :], op0=ALU.mult, op1=ALU.add)
    nc.vector.scalar_tensor_tensor(
        out=ov[:, 1:H, 0:1, :], in0=d[:], scalar=-0.25,
        in1=rh[:, 1:H, :], op0=ALU.mult, op1=ALU.add)
    nc.scalar.copy(ov[:, 0:1, 0:1, :], rh[:, 0:1, :])
    nc.scalar.copy(ov[:, H - 1:H, 1:2, :], rh[:, H - 1:H, :])


# output phase k (k=0..3) lands at position 4a+2+k: (offset into a, lane)
_PH4 = [(0, 2), (0, 3), (1, 0), (1, 1)]


def _up4_w(nc, pool, q, H, W, out, idx):
    """q [128,H,W] bf16 -> out [128,H,4W], half-pixel bilinear 4x, difference
    form: out[4a+2+k] = q[a] + (2k+1)/8 * d[a]."""
    d = pool.tile([128, H, W - 1], BF16, name=f"u4wd_{idx}", tag="u4d")
    nc.vector.tensor_sub(d[:], q[:, :, 1:W], q[:, :, 0:W - 1])
    ov = out.rearrange("p h (a four) -> p h a four", four=4)
    for k, (off, lane) in enumerate(_PH4):
        nc.vector.scalar_tensor_tensor(
            out=ov[:, :, off:off + W - 1, lane:lane + 1],
            in0=d[:], scalar=(2 * k + 1) / 8.0,
            in1=q[:, :, 0:W - 1], op0=ALU.mult, op1=ALU.add)
    nc.scalar.copy(ov[:, :, 0:1, 0:1], q[:, :, 0:1])
    nc.scalar.copy(ov[:, :, 0:1, 1:2], q[:, :, 0:1])
    nc.scalar.copy(ov[:, :, W - 1:W, 2:3], q[:, :, W - 1:W])
    nc.scalar.copy(ov[:, :, W - 1:W, 3:4], q[:, :, W - 1:W])


def _up4_h(nc, pool, rh, H, W2, out, idx):
    """rh [128,H,W2] bf16 -> out [128,4H,W2] (out may be fp8). Two of the
    four phases run fully on GPSIMD (tsmul + add; it has no stt)."""
    d = pool.tile([128, H - 1, W2], BF16, name=f"u4hd_{idx}", tag="u4dh")
    nc.vector.tensor_sub(d[:], rh[:, 1:H, :], rh[:, 0:H - 1, :])
    ov = out.rearrange("p (a four) w -> p a four w", four=4)
    for k, (off, lane) in enumerate(_PH4):
        nc.vector.scalar_tensor_tensor(
            out=ov[:, off:off + H - 1, lane:lane + 1, :],
            in0=d[:], scalar=(2 * k + 1) / 8.0,
            in1=rh[:, 0:H - 1, :], op0=ALU.mult, op1=ALU.add)
    nc.scalar.copy(ov[:, 0:1, 0:1, :], rh[:, 0:1, :])
    nc.scalar.copy(ov[:, 0:1, 1:2, :], rh[:, 0:1, :])
    nc.scalar.copy(ov[:, H - 1:H, 2:3, :], rh[:, H - 1:H, :])
    nc.scalar.copy(ov[:, H - 1:H, 3:4, :], rh[:, H - 1:H, :])


def _stage_a(nc, tc, p1, p2, p3, xcres, wt3_sb, phi3, rep=0):
    """Fill xcres[:, 0:6, :] (fp8, resident) and phi3[m] (fp8 DRAM)."""
    with tc.tile_pool(name=f"sa{rep}", bufs=1) as pool, \
         tc.tile_pool(name=f"ps3{rep}", bufs=4, space="PSUM") as ps3p:
        q3_sb = pool.tile([128, 8, 196], BF16, name=f"q3_{rep}", tag="q3")

        def conv(x, n, idx):
            xb = pool.tile([128, n], BF16, name=f"xb_{idx}", tag="xbf",
                           bufs=2)
            nc.scalar.mul(xb[:], x[:], X_SCALE)
            return xb

        def p3_pool(c):
            x = pool.tile([128, 196], F32, name=f"x3_{c}", tag="xload23",
                          bufs=2)
            nc.gpsimd.dma_start(x[:], p3.ap()[c * 128:(c + 1) * 128, :])
            xb = conv(x, 196, f"p3_{c}")
            _pool3(nc, pool,
                   xb[:].rearrange("p (h w) -> p h w", w=14), 14, 14,
                   q3_sb[:, c, :].rearrange("p (h w) -> p h w", w=14),
                   f"p3_{c}")

        def p1_chunk(c):
            x = pool.tile([128, 3136], F32, name=f"x1_{c}", tag="xload1")
            nc.gpsimd.dma_start(x[:], p1.ap()[c * 128:(c + 1) * 128, :])
            xb = conv(x, 3136, f"p1_{c}")
            _pool3(nc, pool,
                   xb[:].rearrange("p (h w) -> p h w", w=56), 56, 56,
                   xcres[:, c, :].rearrange("p (h w) -> p h w", w=56),
                   f"p1_{c}", final_eng=nc.gpsimd)

        def p2_chunk(c):
            x = pool.tile([128, 784], F32, name=f"x2_{c}", tag="xload23",
                          bufs=2)
            nc.gpsimd.dma_start(x[:], p2.ap()[c * 128:(c + 1) * 128, :])
            xb = conv(x, 784, f"p2_{c}")
            q = pool.tile([128, 28, 28], BF16, name=f"q2_{c}", tag="q")
            _pool3(nc, pool, xb[:].rearrange("p (h w) -> p h w", w=28),
                   28, 28, q[:], f"p2_{c}")
            rh = pool.tile([128, 28, 56], BF16, name=f"rh2_{c}", tag="rh")
            _up2_w(nc, pool, q[:], 28, 28, rh[:], f"p2_{c}")
            _up2_h(nc, pool, rh[:], 28, 56,
                   xcres[:, 2 + c, :].rearrange("p (h w) -> p h w", w=56),
                   f"p2_{c}")

        def psi_step(m):
            # psi3 = W3^T q3 (PE), then 4x upsample -> phi3 DRAM (fp8)
            ps3 = ps3p.tile([128, 196], F32, name=f"ps3_{m}", tag="ps3",
                            padded_shape=[128, 512])
            msl = slice(m * 128, (m + 1) * 128)
            for k8 in range(8):
                nc.tensor.matmul(ps3[:], wt3_sb[:, k8, msl],
                                 q3_sb[:, k8, :],
                                 start=(k8 == 0), stop=(k8 == 7))
            psi = pool.tile([128, 196], BF16, name=f"psi_{m}", tag="psi",
                            bufs=3)
            nc.scalar.copy(psi[:], ps3[:])
            rh3 = pool.tile([128, 14, 56], BF16, name=f"rh3_{m}", tag="rh3",
                            bufs=3)
            _up4_w(nc, pool, psi[:].rearrange("p (h w) -> p h w", w=14),
                   14, 14, rh3[:], f"ps_{m}")
            phi3m = pool.tile([128, 56, 56], F8, name=f"phi3_{m}",
                              tag="phi3m", bufs=3)
            _up4_h(nc, pool, rh3[:], 14, 56, phi3m[:], f"ps_{m}")
            nc.sync.dma_start(
                phi3.ap()[m], phi3m[:].rearrange("p h w -> p (h w)"))

        # Interleave the psi chain between the p1/p2 chunks so phi3 is done
        # by the time xc completes and mm1 can start immediately after.
        for c in range(8):
            p3_pool(c)
        psi_step(0); psi_step(1)
        p1_chunk(0)
        psi_step(2); psi_step(3)
        p1_chunk(1)
        psi_step(4); psi_step(5)
        p2_chunk(0)
        psi_step(6); psi_step(7)
        p2_chunk(1)
        psi_step(8); psi_step(9)
        p2_chunk(2)
        psi_step(10); psi_step(11)
        p2_chunk(3)
        psi_step(12); psi_step(13)


# ------------------------------------------------------------ full program ---
def build_program(debug_xc=False, repeat=1):
    nc = bacc.Bacc("TRN2", target_bir_lowering=False, debug=False,
                   num_devices=N_CORES)

    p1 = nc.dram_tensor("p1", (256, HW), F32, kind="ExternalInput")
    p2 = nc.dram_tensor("p2", (512, 784), F32, kind="ExternalInput")
    p3 = nc.dram_tensor("p3", (1024, 196), F32, kind="ExternalInput")
    wtp = nc.dram_tensor("wtp", (6, 128, CO), F8, kind="ExternalInput")
    wt3 = nc.dram_tensor("wt3", (8, 128, CO), BF16, kind="ExternalInput")
    # rows: [wx, wy, bias] * S_PHI; pairs with coords rows [xx, yy, 1]
    wt2 = nc.dram_tensor("wt2", (3, CO), BF16, kind="ExternalInput")
    ident = nc.dram_tensor("ident", (128, 128), F8, kind="ExternalInput")
    c2 = nc.dram_tensor("c2", (MCH, 128, NCEN), F8, kind="ExternalInput")
    caug = nc.dram_tensor("caug", (4, NCEN), BF16, kind="ExternalInput")
    coords = nc.dram_tensor("coords", (3, HW), BF16, kind="ExternalInput")
    score = nc.dram_tensor("score", (HW,), F32, kind="ExternalOutput")
    phi3s = []
    for r in range(repeat):
        kind = ("ExternalOutput" if (debug_xc and r == 0) else "Internal")
        phi3s.append(nc.dram_tensor(f"phi3_{r}" if r else "phi3",
                                    (MCH, 128, HW), F8, kind=kind))
    scores = [nc.dram_tensor(f"score_dummy{r}", (HW,), F32, kind="Internal")
              for r in range(repeat - 1)] + [score]

    with tile.TileContext(nc) as tc:
        with tc.tile_pool(name="persist", bufs=1) as pp:
            wt3_sb = pp.tile([128, 8, CO], BF16, name="wt3_sb")
            # W3 chunks first: the psi chain consumes them ~10us into the run
            nc.sync.dma_start(wt3_sb[:],
                              wt3.ap().rearrange("k p o -> p k o"))
            wtp_sb = pp.tile([128, 6, CO], F8, name="wtp_sb")
            nc.sync.dma_start(wtp_sb[:],
                              wtp.ap().rearrange("k p o -> p k o"))
            wt2_sb = pp.tile([3, CO], BF16, name="wt2_sb")
            nc.sync.dma_start(wt2_sb[:], wt2.ap())
            coords_sb = pp.tile([3, HW], BF16, name="coords_sb")
            nc.sync.dma_start(coords_sb[:], coords.ap())
            ident_sb = pp.tile([128, 128], F8, name="ident_sb")
            nc.sync.dma_start(ident_sb[:], ident.ap())
            caug_sb = pp.tile([4, NCEN], BF16, name="caug_sb")
            nc.sync.dma_start(caug_sb[:], caug.ap())
            # DoubleRow stationary for the |phi|^2 rows: value 32 so
            # Square(phi/4)*32 = S_C/S_PHI * (S_PHI*phi)^2 = SS*phi^2.
            # Padded to 16 cols so the pair-plane stride is 16B-aligned.
            ones2 = pp.tile([128, 2, 16], F8, name="ones2")
            nc.vector.memset(ones2[:], S_C / S_PHI * 16.0)
            score_sb = pp.tile([128, 25], F32, name="score_sb")
            # first part of the centroid bank loads during stage A; the rest
            # streams in once the stage-A pool frees its SBUF range. Split on
            # an even chunk boundary so fp8 DoubleRow pair-slices never
            # straddle the two tiles.
            c2a_sb = pp.tile([128, 8, NCEN], F8, name="c2a_sb")
            nc.sync.dma_start(
                c2a_sb[:], c2.ap()[0:8].rearrange("k p j -> p k j"))
            # resident fp8 xc, double-buffered by rep parity so rep r+1's
            # stage A overlaps rep r's mm phase
            xcres = [pp.tile([128, 6, HW], F8, name=f"xcres{par}")
                     for par in range(min(repeat, 2))]

            for rep in range(repeat):
                _run_once(nc, tc, rep, p1, p2, p3, coords_sb, c2,
                          xcres[rep % len(xcres)], phi3s[rep], scores[rep],
                          wtp_sb, wt3_sb, wt2_sb, ident_sb, caug_sb, ones2,
                          score_sb, c2a_sb)

    nc.compile()
    return nc


def _run_once(nc, tc, rep, p1, p2, p3, coords_sb, c2, xcres, phi3, score,
              wtp_sb, wt3_sb, wt2_sb, ident_sb, caug_sb, ones2, score_sb,
              c2a_sb):
    _stage_a(nc, tc, p1, p2, p3, xcres, wt3_sb, phi3, rep)

    with tc.tile_pool(name=f"mm{rep}", bufs=2) as mp, \
         tc.tile_pool(name=f"tail{rep}", bufs=2) as tp, \
         tc.tile_pool(name=f"ps_phi{rep}", bufs=2, space="PSUM") as pph, \
         tc.tile_pool(name=f"ps_n{rep}", bufs=2, space="PSUM") as pn, \
         tc.tile_pool(name=f"ps_d{rep}", bufs=3, space="PSUM") as pd:
        c2b_sb = mp.tile([128, MCH - 8, NCEN], F8, name="c2b_sb", bufs=1)
        nc.sync.dma_start(
            c2b_sb[:], c2.ap()[8:MCH].rearrange("k p j -> p k j"))

        def c2_pair(k2, jsl):
            # [128, 2, wj] moving AP for DoubleRow pair k2 (channel blocks
            # 2*k2 and 2*k2+1).
            return (c2a_sb[:, 2 * k2:2 * k2 + 2, jsl] if k2 < 4
                    else c2b_sb[:, 2 * k2 - 8:2 * k2 - 6, jsl])

        top8_all = tp.tile([128, 25, 8], F32, name=f"t8a_{rep}",
                           tag="t8all", bufs=1)
        nc.vector.memset(top8_all[:], 0.0)
        for ci, (c0, w) in enumerate(HW_CHUNKS):
            sl = slice(c0, c0 + w)
            phi3_t = mp.tile([128, MCH, w], F8, name=f"p3t_{ci}",
                             tag="phi3t", padded_shape=[128, MCH, 512],
                             bufs=2)
            nc.sync.dma_start(
                phi3_t[:],
                phi3.ap()[:, :, sl].rearrange("k p n -> p k n"))
            # ---- mm1 (phi = W^T xc + b + phi3): fp8 DoubleRow pairs over
            # the 6 xc chunks, K=2 bf16 for coords; eviction on GPSIMD; the
            # |phi|^2 ones-matmuls (fp8 DoubleRow on Square(phi/4)) trail
            # one pair behind.
            phi_sb = mp.tile([128, MCH, w], F8, name=f"phi_{ci}",
                             tag="phi", padded_shape=[128, MCH, 512])
            phisq = mp.tile([128, MCH, w], F8, name=f"phsq_{ci}",
                            tag="phisq", padded_shape=[128, MCH, 512])
            ps_n = pn.tile([2, w], F32, name=f"psn_{ci}", tag="psn",
                           padded_shape=[2, 512])

            def emit_ones_mm(k2):
                nc.tensor.matmul(ps_n[:], ones2[:, :, 0:2],
                                 phisq[:, 2 * k2:2 * k2 + 2, :],
                                 start=(k2 == 0), stop=(k2 == MCH // 2 - 1),
                                 perf_mode=DR, skip_group_check=True)

            for m in range(MCH):
                ph = pph.tile([128, w], F32, name=f"ph_{ci}_{m}",
                              tag="ph", padded_shape=[128, 512])
                msl = slice(m * 128, (m + 1) * 128)
                for kk in range(3):
                    nc.tensor.matmul(ph[:],
                                     wtp_sb[:, 2 * kk:2 * kk + 2, msl],
                                     xcres[:, 2 * kk:2 * kk + 2, sl],
                                     start=(kk == 0), stop=False,
                                     perf_mode=DR)
                # coords + bias rows (K=3, bf16)
                nc.tensor.matmul(ph[:], wt2_sb[:, msl], coords_sb[:, sl],
                                 start=False, stop=False)
                # fold phi3 into PSUM via an fp8 identity matmul, so the
                # eviction is a single ACT copy and Square reads PSUM too
                # (keeps DVE out of the mm1 pipeline entirely).
                nc.tensor.matmul(ph[:], ident_sb[:], phi3_t[:, m, :],
                                 start=False, stop=True)
                nc.scalar.copy(phi_sb[:, m, :], ph[:])
                nc.scalar.activation(phisq[:, m, :], ph[:],
                                     AF.Square, scale=0.25)
                if m >= 3 and m % 2 == 1:
                    emit_ones_mm((m - 3) // 2)
            emit_ones_mm(MCH // 2 - 1)
            # aug rows: [hi, lo, 1, 1] of SS*|phi|^2 built from ps_n (rows
            # 0,1 of ps_n are identical). Compute-ops may not address
            # partition base 1, so the lo row is placed via a tiny DMA.
            aug = mp.tile([4, w], BF16, name=f"aug_{ci}", tag="aug",
                          padded_shape=[4, 512], bufs=2)
            nc.vector.memset(aug[:], 1.0)
            nc.scalar.copy(aug[0:2, :], ps_n[0:2, :])
            lo2 = mp.tile([2, w], BF16, name=f"lo2_{ci}", tag="lo2",
                          padded_shape=[2, 512], bufs=2)
            nc.vector.tensor_sub(lo2[:], ps_n[0:2, :], aug[0:2, :])
            nc.sync.dma_start(aug[1:2, :], lo2[1:2, :])
            # ---- mm2 + top-k per 128-row block, all fp8 DoubleRow
            nblk = (w + 127) // 128
            for blk in range(nblk):
                mblk = min(128, w - blk * 128)
                bsl = slice(blk * 128, blk * 128 + mblk)
                m8 = tp.tile([128, 56], F32, name=f"m8_{ci}_{blk}",
                             tag="m8", bufs=2)
                for j, (j0, wj) in enumerate(CEN_CHUNKS):
                    jsl = slice(j0, j0 + wj)
                    pdt = pd.tile([128, wj], F32,
                                  name=f"pd_{ci}_{blk}_{j}", tag="pd",
                                  padded_shape=[128, 512])
                    for k2 in range(MCH // 2):
                        nc.tensor.matmul(pdt[0:mblk, :],
                                         phi_sb[:, 2 * k2:2 * k2 + 2, bsl],
                                         c2_pair(k2, jsl),
                                         start=(k2 == 0), stop=False,
                                         perf_mode=DR)
                    nc.tensor.matmul(pdt[0:mblk, :], aug[:, bsl],
                                     caug_sb[:, jsl],
                                     start=False, stop=True)
                    nc.vector.max(out=m8[0:mblk, j * 8:(j + 1) * 8],
                                  in_=pdt[0:mblk, :])
                g = ci * 4 + blk
                nc.vector.max(out=top8_all[0:mblk, g, :],
                              in_=m8[0:mblk, :])
        # ---- batched tail over all 25 row-blocks at once
        v3a = tp.tile([128, 25, 3], F32, name=f"v3a_{rep}", tag="v3a", bufs=1)
        nc.scalar.activation(v3a[:], top8_all[:, :, 0:3], AF.Sqrt,
                             scale=-1.0 / SS)
        e3a = tp.tile([128, 25, 3], F32, name=f"e3a_{rep}", tag="e3a", bufs=1)
        nc.scalar.activation(e3a[:], v3a[:], AF.Exp, scale=-1.0)
        den = tp.tile([128, 25], F32, name=f"den_{rep}", tag="den", bufs=1)
        nc.vector.reduce_sum(den[:], e3a[:], axis=AX.X)
        rec = tp.tile([128, 25], F32, name=f"rec_{rep}", tag="rec", bufs=1)
        nc.vector.reciprocal(rec[:], den[:])
        num = tp.tile([128, 25], F32, name=f"num_{rep}", tag="num", bufs=1)
        nc.vector.tensor_mul(num[:], e3a[:, :, 0], v3a[:, :, 0])
        nc.vector.tensor_mul(score_sb[:, 0:25], num[:], rec[:])
        nc.sync.dma_start(
            score.ap()[0:3072].rearrange("(n p) -> p n", p=128),
            score_sb[:, 0:24])
        nc.sync.dma_start(
            score.ap()[3072:3136].rearrange("(n p) -> p n", p=64),
            score_sb[0:64, 24:25])


# ------------------------------------------------------------- host side ----
def prep_shared(W, b, C):
    W = np.asarray(W, np.float32)
    b = np.asarray(b, np.float32)
    C = np.asarray(C, np.float32)
    # inputs carry X_SCALE=1/4; weights carry S_W8=64 -> phi is 16x.
    wt_full = W[:, :CO].T / 9.0 * S_W8                     # (1792, 1792)
    wtp = np.ascontiguousarray(
        wt_full[:768].reshape(6, 128, CO)).astype(ml_dtypes.float8_e4m3)
    wt3 = np.ascontiguousarray(
        wt_full[768:].reshape(8, 128, CO)).astype(ml_dtypes.bfloat16)
    wt2 = np.ascontiguousarray(np.concatenate(
        [W[:, CO:CO + 2].T, b[None, :]]) * S_PHI).astype(ml_dtypes.bfloat16)
    ident = np.eye(128, dtype=ml_dtypes.float8_e4m3)
    c2f8 = (2.0 * S_C * C).astype(ml_dtypes.float8_e4m3)
    c2 = np.ascontiguousarray(c2f8.reshape(MCH, 128, NCEN))
    # |C|^2 of the fp8-quantized centroids at the SS PSUM scale.
    cq = c2f8.astype(np.float64) / (2.0 * S_C)
    cn = SS * (cq ** 2).sum(axis=0)
    hi = cn.astype(ml_dtypes.bfloat16)
    lo = (cn - hi.astype(np.float64)).astype(ml_dtypes.bfloat16)
    caug = np.stack([
        -np.ones(NCEN, ml_dtypes.bfloat16),
        -np.ones(NCEN, ml_dtypes.bfloat16),
        -hi, -lo]).astype(ml_dtypes.bfloat16)
    lin = np.linspace(-1.0, 1.0, 56, dtype=np.float32)
    xx = np.broadcast_to(lin[None, :], (56, 56)).reshape(HW)
    yy = np.broadcast_to(lin[:, None], (56, 56)).reshape(HW)
    coords = np.stack([xx, yy, np.ones(HW, np.float32)]).astype(
        ml_dtypes.bfloat16)
    return {"wtp": wtp, "wt3": wt3, "wt2": wt2, "ident": ident, "c2": c2,
            "caug": caug, "coords": coords}


def make_in_maps(p1, p2, p3, W, b, C):
    shared = prep_shared(W, b, C)
    maps = []
    for i in range(N_CORES):
        m = dict(shared)
        m["p1"] = np.ascontiguousarray(np.asarray(p1)[i], np.float32).reshape(256, HW)
        m["p2"] = np.ascontiguousarray(np.asarray(p2)[i], np.float32).reshape(512, 784)
        m["p3"] = np.ascontiguousarray(np.asarray(p3)[i], np.float32).reshape(1024, 196)
        maps.append(m)
    return maps


# ------------------------------------------------------------- runner -------
class _Runtime:
    def __init__(self, repeat=1):
        import jax
        from jax.sharding import Mesh, PartitionSpec
        from jax.experimental.shard_map import shard_map
        from concourse import bass2jax

        self.jax = jax
        self.nc = build_program(repeat=repeat)
        nc = self.nc
        bass2jax.install_neuronx_cc_hook()

        partition_name = (nc.partition_id_tensor.name
                          if nc.partition_id_tensor else None)
        in_names, out_names, out_avals, zero_outs = [], [], [], []
        for alloc in nc.m.functions[0].allocations:
            if not isinstance(alloc, mybir.MemoryLocationSet):
                continue
            name = alloc.memorylocations[0].name
            if alloc.kind == "ExternalInput":
                if name != partition_name:
                    in_names.append(name)
            elif alloc.kind == "ExternalOutput":
                shape = tuple(alloc.tensor_shape)
                dtype = mybir.dt.np(alloc.dtype)
                out_names.append(name)
                out_avals.append(jax.core.ShapedArray(shape, dtype))
                zero_outs.append(np.zeros(shape, dtype))
        self.in_names = list(in_names)
        self.out_names = out_names
        self.out_avals = out_avals
        self.zero_outs = zero_outs
        n_params = len(in_names)
        n_outs = len(out_avals)
        all_in_names = in_names + out_names
        if partition_name is not None:
            all_in_names.append(partition_name)

        def _body(*args):
            operands = list(args)
            if partition_name is not None:
                operands.append(bass2jax.partition_id_tensor())
            outs = bass2jax._bass_exec_p.bind(
                *operands,
                out_avals=tuple(out_avals),
                in_names=tuple(all_in_names),
                out_names=tuple(out_names),
                lowering_input_output_aliases=(),
                sim_require_finite=True,
                sim_require_nnan=True,
                nc=nc,
            )
            return tuple(outs)

        devices = jax.devices()[:N_CORES]
        mesh = Mesh(np.asarray(devices), ("core",))
        self.mesh = mesh
        self.pspec = PartitionSpec("core")
        in_specs = (PartitionSpec("core"),) * (n_params + n_outs)
        out_specs = (PartitionSpec("core"),) * n_outs
        self.sharded = jax.jit(
            shard_map(_body, mesh=mesh, in_specs=in_specs,
                      out_specs=out_specs, check_rep=False),
            donate_argnums=tuple(range(n_params, n_params + n_outs)),
            keep_unused=True,
        )

    def concat_inputs(self, in_maps):
        return [np.concatenate([np.asarray(in_maps[c][nm])
                                for c in range(N_CORES)], axis=0)
                for nm in self.in_names]

    def zeros(self):
        return [np.zeros((N_CORES * z.shape[0], *z.shape[1:]), z.dtype)
                for z in self.zero_outs]

    def device_put_sharded(self, arrays):
        from jax.sharding import NamedSharding
        sh = NamedSharding(self.mesh, self.pspec)
        return [self.jax.device_put(a, sh) for a in arrays]

    def run(self, in_maps):
        outs = self.sharded(*self.concat_inputs(in_maps), *self.zeros())
        res = {}
        for i, nm in enumerate(self.out_names):
            a = np.asarray(outs[i])
            res[nm] = a.reshape(N_CORES, *self.out_avals[i].shape)
        return res


_RT = {}


def _runtime(repeat=1):
    if repeat not in _RT:
        _RT[repeat] = _Runtime(repeat=repeat)
    return _RT[repeat]


def kernel(p1, p2, p3, W, b, C):
    rt = _runtime()
    in_maps = make_in_maps(p1, p2, p3, W, b, C)
    res = rt.run(in_maps)
    return np.ascontiguousarray(
        res["score"].reshape(N_CORES, 1, 56, 56).astype(np.float32))



# revision 61
# speedup vs baseline: 1.7552x; 1.7552x over previous
"""DSVDD retrieval-knn kernel for 8 Trainium2 NeuronCores (v2).

Data-parallel over batch: one image per NeuronCore, no collectives.
Per image, per rep:

  stage A  p1/p2/p3 arrive pre-scaled (0.25x) in bf16 from the host.
           3x3 sum-pools run as batched DVE tensor_add chains (bf16 2x
           mode); p2 is 2x-bilinear-upsampled in difference form.
           p1/p2 land in a resident fp8 xc tile.  p3 is pooled at 14x14
           (fp8 q3), then psi^T = (W3^T q3)^T is computed DIRECTLY
           transposed on PE (q3 stationary, fp8 DoubleRow) into a
           resident fp8 psiT tile -- no upsampling on DVE, no DRAM
           roundtrip.
  mm1      phi = W12^T xc + [U | coords]-pair @ psiT_ext, all fp8
           DoubleRow (4 matmuls per 128-row block): the 4x bilinear
           14->56 upsample of psi is folded into a single DR pair via a
           static fp8 interpolation matrix U (196 source positions =
           2 planes x 98 partitions), with the CoordConv rows riding in
           partitions 98-100 of the same pair.  PSUM eviction is one
           ACT copy -> fp8 phi.
  mm2      PSUM = 2*SS*phi.C - SS*|C|^2 via 7 fp8 DR pairs + one K=1
           fp8 DR "bias pair" (|C|^2 split hi/lo at scales 64/8).  The
           row-constant -SS*|phi|^2 term is NOT added before top-k (it
           cannot change the per-row ranking); instead 16*|phi|^2 comes
           from a small phi^T phi DR matmul per row-block whose diagonal
           is extracted by bouncing the [128,128] tile through DRAM with
           row stride 129 and reading back at stride 130.
  top-3    vector.max (top-8) per 448-column PSUM block + merge, then
           vals = sqrt((SS*|phi|^2 - top3)/SS); softmin -> w0*v0,
           batched over all 25 row-blocks.

Reps are software-pipelined: stage A of rep r+1 is EMITTED interleaved
with the mm chunks of rep r (engines execute in program order), so the
DVE-heavy stage A hides under the PE-bound mm phase in steady state.
"""
import os, sys

sys.path.insert(0, os.environ.get("TRN_RL_REPO", "/opt/trn_rl_repo"))

import numpy as np
import ml_dtypes

import concourse.bass as bass
import concourse.tile as tile
from concourse import bacc, mybir

F32 = mybir.dt.float32
BF16 = mybir.dt.bfloat16
F8 = mybir.dt.float8e4
AF = mybir.ActivationFunctionType
ALU = mybir.AluOpType
AX = mybir.AxisListType
DR = mybir.MatmulPerfMode.DoubleRow

N_CORES = 8
HW = 3136          # 56*56
NCEN = 3136        # centroids
CO = 1792          # output channels / feature dim
MCH = 14           # CO / 128
HW_CHUNKS = [(i * 512, 512) for i in range(6)] + [(3072, 64)]
CEN_CHUNKS = [(i * 448, 448) for i in range(7)]
# Scale plan (exact powers of two): inputs carry 0.25 (host bf16 cast);
# fp8 weights carry 64/9 -> phi PSUM is 16*phi, evicted fp8 phi is
# 4*phi (ACT eviction scale 0.25, so Pool's phisq = (4phi)^2 = 16phi^2
# stays in fp8 range).  Centroids are fp8(64*C).  mm2 PSUM =
# 2*SS*phi.C - SS*|C|^2 with SS = 4*32 = 128.
S_PHI = 16.0
PHI8 = 4.0
S_C = 32.0
SS = PHI8 * S_C
S_W8 = 64.0
X_SCALE = 0.25

_DBG = set(os.environ.get("KDBG", "").split(","))


# ---------------------------------------------------------------- stage A ---
def _up2_w(nc, pool, q, H, W, out, idx):
    """q [128,H,W] bf16 -> out [128,H,2W], half-pixel 2x bilinear on the
    last axis in difference form."""
    d = pool.tile([128, H, W - 1], BF16, name=f"u2wd_{idx}", tag="u2d")
    nc.vector.tensor_sub(d[:], q[:, :, 1:W], q[:, :, 0:W - 1])
    ov = out.rearrange("p h (a two) -> p h a two", two=2)
    nc.vector.scalar_tensor_tensor(
        out=ov[:, :, 0:W - 1, 1:2], in0=d[:], scalar=0.25,
        in1=q[:, :, 0:W - 1], op0=ALU.mult, op1=ALU.add)
    nc.vector.scalar_tensor_tensor(
        out=ov[:, :, 1:W, 0:1], in0=d[:], scalar=-0.25,
        in1=q[:, :, 1:W], op0=ALU.mult, op1=ALU.add)
    nc.scalar.copy(ov[:, :, 0:1, 0:1], q[:, :, 0:1])
    nc.scalar.copy(ov[:, :, W - 1:W, 1:2], q[:, :, W - 1:W])


def _up2_h(nc, pool, rh, H, W2, out, idx):
    """rh [128,H,W2] bf16 -> out [128,2H,W2] (fp8 ok)."""
    d = pool.tile([128, H - 1, W2], BF16, name=f"u2hd_{idx}", tag="u2dh")
    nc.vector.tensor_sub(d[:], rh[:, 1:H, :], rh[:, 0:H - 1, :])
    ov = out.rearrange("p (a two) w -> p a two w", two=2)
    nc.vector.scalar_tensor_tensor(
        out=ov[:, 0:H - 1, 1:2, :], in0=d[:], scalar=0.25,
        in1=rh[:, 0:H - 1, :], op0=ALU.mult, op1=ALU.add)
    nc.vector.scalar_tensor_tensor(
        out=ov[:, 1:H, 0:1, :], in0=d[:], scalar=-0.25,
        in1=rh[:, 1:H, :], op0=ALU.mult, op1=ALU.add)
    nc.scalar.copy(ov[:, 0:1, 0:1, :], rh[:, 0:1, :])
    nc.scalar.copy(ov[:, H - 1:H, 1:2, :], rh[:, H - 1:H, :])


# ------------------------------------------------------------ full program ---
def build_program(repeat=1):
    nc = bacc.Bacc("TRN2", target_bir_lowering=False, debug=False,
                   num_devices=N_CORES)

    p1 = nc.dram_tensor("p1", (256, HW), BF16, kind="ExternalInput")
    p2 = nc.dram_tensor("p2", (512, 784), BF16, kind="ExternalInput")
    p3 = nc.dram_tensor("p3", (1024, 196), BF16, kind="ExternalInput")
    wtp = nc.dram_tensor("wtp", (6, 128, CO), F8, kind="ExternalInput")
    wt3 = nc.dram_tensor("wt3", (8, 128, CO), F8, kind="ExternalInput")
    wco = nc.dram_tensor("wco", (3, CO), F8, kind="ExternalInput")
    ucoord = nc.dram_tensor("ucoord", (128, 2, NCEN), F8,
                            kind="ExternalInput")
    c2 = nc.dram_tensor("c2", (MCH, 128, NCEN), F8, kind="ExternalInput")
    cbx = nc.dram_tensor("cbx", (1, 2 * NCEN), F8, kind="ExternalInput")
    stb = nc.dram_tensor("stb", (1, 256), F8, kind="ExternalInput")
    dg_dram = [nc.dram_tensor(f"dg_dram{i}", (128 * 130,), F32,
                               kind="Internal") for i in range(2)]
    score = nc.dram_tensor("score", (HW,), F32, kind="ExternalOutput")
    scores = [nc.dram_tensor(f"score_dummy{r}", (HW,), F32, kind="Internal")
              for r in range(repeat - 1)] + [score]

    npar = min(repeat, 2)
    with tile.TileContext(nc) as tc:
        with tc.tile_pool(name="persist", bufs=1) as pp, \
             tc.tile_pool(name="sa", bufs=1) as sp, \
             tc.tile_pool(name="mm", bufs=1) as mp, \
             tc.tile_pool(name="ps_phi", bufs=2, space="PSUM") as pph, \
             tc.tile_pool(name="ps_d", bufs=3, space="PSUM") as pd, \
             tc.tile_pool(name="ps_n", bufs=1, space="PSUM") as pn, \
             tc.tile_pool(name="ps_t", bufs=2, space="PSUM") as pst:
            g = {}
            g["wt3_sb"] = pp.tile([128, 8, CO], F8, name="wt3_sb")
            nc.sync.dma_start(g["wt3_sb"][:],
                              wt3.ap().rearrange("k p o -> p k o"))
            g["wtp_sb"] = pp.tile([128, 6, CO], F8, name="wtp_sb")
            nc.sync.dma_start(g["wtp_sb"][:],
                              wtp.ap().rearrange("k p o -> p k o"))
            g["uc_sb"] = pp.tile([128, 2, NCEN], F8, name="uc_sb")
            nc.sync.dma_start(g["uc_sb"][:], ucoord.ap())
            g["c2_sb"] = pp.tile([128, MCH, NCEN], F8, name="c2_sb")
            nc.sync.dma_start(g["c2_sb"][:],
                              c2.ap().rearrange("k p j -> p k j"))
            g["cbx_sb"] = pp.tile([1, 2, NCEN], F8, name="cbx_sb")
            nc.sync.dma_start(
                g["cbx_sb"][:].rearrange("p two n -> p (two n)"), cbx.ap())
            g["stb_sb"] = pp.tile([1, 2, 128], F8, name="stb_sb")
            nc.sync.dma_start(
                g["stb_sb"][:].rearrange("p two n -> p (two n)"), stb.ap())
            # per-parity dynamic residents
            g["xcres"] = [pp.tile([128, 6, NCEN], F8, name=f"xcres{par}")
                          for par in range(npar)]
            g["psiT"] = [pp.tile([128, 2, CO], F8, name=f"psiT{par}")
                         for par in range(npar)]
            # q3 halves: [128, h, k, 112] with [.,h,k,0:98] = rows
            # 7h..7h+6 of the pooled 14x14 map (psi stationary pair
            # stride 112B, 16B-aligned)
            g["q3"] = [pp.tile([128, 2, 8, 112], F8, name=f"q3_{par}")
                       for par in range(npar)]
            for par in range(npar):
                nc.vector.memset(g["psiT"][par][:], 0.0)
                nc.sync.dma_start(g["psiT"][par][98:101, 0, :], wco.ap())
            g["top8"] = pp.tile([128, 25, 8], F32, name="top8")
            nc.vector.memset(g["top8"][:], 0.0)
            g["ssphi2"] = pp.tile([128, 25], F32, name="ssphi2")
            nc.vector.memset(g["ssphi2"][:], 0.0)
            g["score_sb"] = pp.tile([128, 25], F32, name="score_sb")

            def sa_queue(rep):
                return [fn() for fn in
                        _sa_slices(nc, sp, pst, g, rep % npar, p1, p2, p3)]

            _pump(sa_queue(0), -1)
            for rep in range(repeat):
                q = sa_queue(rep + 1) if rep + 1 < repeat else []
                filler = lambda: _pump(q, 1)
                mm = _mm_slices(nc, mp, pph, pd, g, rep % npar, rep,
                                dg_dram, filler)
                for m in mm:
                    m()
                _pump(q, -1)
                _tail(nc, mp, g, rep, scores[rep])

    nc.compile()
    return nc


# stage A emitted as a queue of generators; each yield is a resume point
# so the emission can be pumped in fine grains between the previous rep's
# mm2 blocks (engines execute in program order -- coarse stage-A blocks
# would stall the max8 stream and with it PE's PSUM recycling).
def _pool3_gen(nc, pool, xv, B, H, W, out, idx, tags=("pa", "ps", "pt")):
    a = pool.tile([128, B, H, W], BF16, name=f"pa_{idx}", tag=tags[0])
    nc.vector.tensor_add(a[:, :, :, 0:W - 1], xv[:, :, :, 0:W - 1],
                         xv[:, :, :, 1:W])
    nc.scalar.copy(a[:, :, :, W - 1:W], xv[:, :, :, W - 1:W])
    yield
    s = pool.tile([128, B, H, W], BF16, name=f"ps_{idx}", tag=tags[1])
    nc.vector.tensor_add(s[:, :, :, 1:W], a[:, :, :, 1:W],
                         xv[:, :, :, 0:W - 1])
    nc.scalar.copy(s[:, :, :, 0:1], a[:, :, :, 0:1])
    yield
    t = pool.tile([128, B, H, W], BF16, name=f"pt_{idx}", tag=tags[2])
    nc.vector.tensor_add(t[:, :, 0:H - 1, :], s[:, :, 0:H - 1, :],
                         s[:, :, 1:H, :])
    nc.scalar.copy(t[:, :, H - 1:H, :], s[:, :, H - 1:H, :])
    yield
    if out is None:
        yield (t, s)
        return
    nc.vector.tensor_add(out[:, :, 1:H, :], t[:, :, 1:H, :],
                         s[:, :, 0:H - 1, :])
    nc.scalar.copy(out[:, :, 0:1, :], t[:, :, 0:1, :])


def _sa_slices(nc, sp, pst, g, par, p1, p2, p3):
    xcres, psiT, q3 = g["xcres"][par], g["psiT"][par], g["q3"][par]
    wt3_sb = g["wt3_sb"]

    def p3_pool():
        x = sp.tile([128, 3136], BF16, name=f"x3_{par}", tag="xload",
                    bufs=2)
        nc.sync.dma_start(x[:, 0:1568].rearrange("p (k n) -> p k n", n=196),
                          p3.ap().rearrange("(k p) n -> p k n", p=128))
        yield
        xv = x[:, 0:1568].rearrange("p (k h w) -> p k h w", k=8, w=14)
        gen = _pool3_gen(nc, sp, xv, 8, 14, 14, None, f"p3_{par}")
        t = s = None
        for v in gen:
            if v is not None:
                t, s = v
                break
            yield
        # final add, split into the two row-halves of the q3 layout
        h0 = q3[:, 0, :, 0:98].rearrange("p k (r w) -> p k r w", w=14)
        h1 = q3[:, 1, :, 0:98].rearrange("p k (r w) -> p k r w", w=14)
        nc.vector.tensor_add(h0[:, :, 1:7, :], t[:, :, 1:7, :],
                             s[:, :, 0:6, :])
        nc.vector.tensor_add(h1[:], t[:, :, 7:14, :], s[:, :, 6:13, :])
        nc.scalar.copy(h0[:, :, 0:1, :], t[:, :, 0:1, :])

    def psi(m0, m1):
        # psiT[p, h, c] = sum_k q3[k-blocks; pos 98h+p] * wt3[k; c],
        # two 128-col m-blocks per PSUM tile (N=256 moving)
        def f():
            if "nopsi" in _DBG:
                return
            for m in range(m0, m1, 2):
                ps = pst.tile([98, 2, 256], F32, name=f"pst_{par}_{m}",
                              tag="pst", padded_shape=[128, 2, 256])
                msl = slice(m * 128, (m + 2) * 128)
                for h in range(2):
                    for k in range(4):
                        nc.tensor.matmul(
                            ps[:, h, :],
                            q3[:, h, 2 * k:2 * k + 2, 0:98],
                            wt3_sb[:, 2 * k:2 * k + 2, msl],
                            start=(k == 0), stop=(k == 3), perf_mode=DR)
                nc.scalar.copy(psiT[0:98, :, msl], ps[:])
                yield
        return f

    def p1_chunk(c):
        def f():
            x = sp.tile([128, 3136], BF16, name=f"x1_{par}_{c}",
                        tag="xload", bufs=2)
            nc.sync.dma_start(x[:], p1.ap()[c * 128:(c + 1) * 128, :])
            yield
            yield from _pool3_gen(
                nc, sp, x[:].rearrange("p (b h w) -> p b h w", b=1, w=56),
                1, 56, 56,
                xcres[:, c, :].rearrange("p (b h w) -> p b h w", b=1, w=56),
                f"p1_{par}_{c}")
        return f

    def p2_sub(s2):
        def f():
            x = sp.tile([128, 3136], BF16, name=f"x2_{par}_{s2}",
                        tag="xload", bufs=2)
            nc.sync.dma_start(
                x[:, 0:1568].rearrange("p (k n) -> p k n", n=784),
                p2.ap()[s2 * 256:(s2 + 1) * 256, :].rearrange(
                    "(k p) n -> p k n", p=128))
            yield
            xv = x[:, 0:1568].rearrange("p (b h w) -> p b h w", b=2, w=28)
            q = sp.tile([128, 2, 28, 28], BF16, name=f"q2_{par}_{s2}",
                        tag="q2")
            yield from _pool3_gen(nc, sp, xv, 2, 28, 28, q[:],
                                  f"p2_{par}_{s2}")
            yield
            rh = sp.tile([128, 2, 28, 56], BF16, name=f"rh_{par}_{s2}",
                         tag="rh")
            # 2x in w: batched over both chunks (rows stack: 2*28=56)
            _up2_w(nc, sp, q[:].rearrange("p b h w -> p (b h) w"), 56, 28,
                   rh[:].rearrange("p b h w -> p (b h) w"), f"p2_{par}_{s2}")
            yield
            for c in range(2):
                _up2_h(nc, sp, rh[:, c], 28, 56,
                       xcres[:, 2 + 2 * s2 + c, :].rearrange(
                           "p (h w) -> p h w", w=56),
                       f"p2_{par}_{s2}_{c}")
                yield
        return f

    return [p3_pool, psi(0, 8), psi(8, 14), p1_chunk(0), p1_chunk(1),
            p2_sub(0), p2_sub(1)]


def _pump(queue, n):
    """Advance up to n emission steps from the generator queue."""
    done = 0
    while queue and (n < 0 or done < n):
        try:
            next(queue[0])
            done += 1
        except StopIteration:
            queue.pop(0)


def _mm_slices(nc, mp, pph, pd, g, par, rep, dg_dram, filler):
    xcres, psiT = g["xcres"][par], g["psiT"][par]
    wtp_sb, uc_sb = g["wtp_sb"], g["uc_sb"]
    c2_sb, cbx_sb, stb_sb = g["c2_sb"], g["cbx_sb"], g["stb_sb"]
    top8, ssphi2 = g["top8"], g["ssphi2"]

    def chunk(ci):
        c0, w = HW_CHUNKS[ci]
        sl = slice(c0, c0 + w)

        def f():
            phi_sb = mp.tile([128, MCH, w], F8, name=f"phi_{rep}_{ci}",
                             tag="phi", padded_shape=[128, MCH, 512],
                             bufs=2)
            # ---- mm1: 4 fp8 DR pairs per 128-row block
            for m in range(MCH):
                ph = pph.tile([128, w], F32, name=f"ph_{rep}_{ci}_{m}",
                              tag="ph", padded_shape=[128, 512])
                msl = slice(m * 128, (m + 1) * 128)
                for kk in range(3):
                    nc.tensor.matmul(ph[:],
                                     wtp_sb[:, 2 * kk:2 * kk + 2, msl],
                                     xcres[:, 2 * kk:2 * kk + 2, sl],
                                     start=(kk == 0), stop=False,
                                     perf_mode=DR)
                nc.tensor.matmul(ph[:], psiT[:, :, msl], uc_sb[:, :, sl],
                                 start=False, stop=True, perf_mode=DR)
                nc.scalar.mul(phi_sb[:, m, :], ph[:], PHI8 / S_PHI)
                if m % 7 == 3:
                    filler()
            # ---- mm2 + top-8 per 128-row block
            nblk = (w + 127) // 128
            for blk in range(nblk):
                mblk = min(128, w - blk * 128)
                bsl = slice(blk * 128, blk * 128 + mblk)
                gi = ci * 4 + blk
                m8 = mp.tile([128, 56], F32, name=f"m8_{rep}_{ci}_{blk}",
                             tag="m8", bufs=2)
                for j, (j0, wj) in enumerate(CEN_CHUNKS):
                    jsl = slice(j0, j0 + wj)
                    pdt = pd.tile([128, wj], F32,
                                  name=f"pd_{rep}_{ci}_{blk}_{j}", tag="pd",
                                  padded_shape=[128, 512])
                    for k2 in range(MCH // 2):
                        nc.tensor.matmul(
                            pdt[0:mblk, :],
                            phi_sb[:, 2 * k2:2 * k2 + 2, bsl],
                            c2_sb[:, 2 * k2:2 * k2 + 2, jsl],
                            start=(k2 == 0), stop=False, perf_mode=DR)
                    nc.tensor.matmul(pdt[0:mblk, :],
                                     stb_sb[:, :, 0:mblk],
                                     cbx_sb[:, :, jsl],
                                     start=False, stop=True,
                                     perf_mode=DR)
                    nc.vector.max(out=m8[0:mblk, j * 8:(j + 1) * 8],
                                  in_=pdt[0:mblk, :])
                nc.vector.max(out=top8[0:mblk, gi, :], in_=m8[0:mblk, :])
                # 16*|phi|^2 diagonal: phi^T phi (7 DR pairs), evict to
                # SBUF, bounce via DRAM rows of stride 129 so a single
                # stride-130 read lands the diagonal in ssphi2[:, gi]
                pdg = pd.tile([128, 128], F32, name=f"pg_{rep}_{ci}_{blk}",
                              tag="pd", padded_shape=[128, 512])
                for k2 in range(MCH // 2):
                    nc.tensor.matmul(pdg[0:mblk, 0:mblk],
                                     phi_sb[:, 2 * k2:2 * k2 + 2, bsl],
                                     phi_sb[:, 2 * k2:2 * k2 + 2, bsl],
                                     start=(k2 == 0),
                                     stop=(k2 == MCH // 2 - 1),
                                     perf_mode=DR)
                dsc = mp.tile([128, 128], F32, name=f"dsc_{rep}_{ci}_{blk}",
                              tag="dsc", bufs=2)
                nc.scalar.copy(dsc[0:mblk, 0:mblk], pdg[0:mblk, 0:mblk])
                dgv = dg_dram[gi % 2].ap()
                nc.sync.dma_start(
                    dgv[0:mblk * 129].rearrange(
                        "(p q) -> p q", q=129)[:, 0:mblk],
                    dsc[0:mblk, 0:mblk])
                nc.sync.dma_start(
                    ssphi2[0:mblk, gi:gi + 1],
                    dgv[0:mblk * 130].rearrange(
                        "(p q) -> p q", q=130)[:, 0:1])
                filler()
        return f

    return [chunk(ci) for ci in range(len(HW_CHUNKS))]


def _tail(nc, mp, g, rep, score):
    top8, ssphi2, score_sb = g["top8"], g["ssphi2"], g["score_sb"]
    t3m = mp.tile([128, 25, 3], F32, name=f"t3m_{rep}", tag="t3m", bufs=1)
    # ssphi2 holds 16*|phi|^2; SS*|phi|^2 - top8 = 8*ssphi2 - top8
    for k in range(3):
        nc.vector.scalar_tensor_tensor(
            out=t3m[:, :, k], in0=ssphi2[:], scalar=SS / S_PHI,
            in1=top8[:, :, k], op0=ALU.mult, op1=ALU.subtract)
    v3a = mp.tile([128, 25, 3], F32, name=f"v3a_{rep}", tag="v3a", bufs=1)
    nc.scalar.activation(v3a[:], t3m[:], AF.Sqrt, scale=1.0 / SS)
    e3a = mp.tile([128, 25, 3], F32, name=f"e3a_{rep}", tag="e3a", bufs=1)
    nc.scalar.activation(e3a[:], v3a[:], AF.Exp, scale=-1.0)
    den = mp.tile([128, 25], F32, name=f"den_{rep}", tag="den", bufs=1)
    nc.vector.reduce_sum(den[:], e3a[:], axis=AX.X)
    rec = mp.tile([128, 25], F32, name=f"rec_{rep}", tag="rec", bufs=1)
    nc.vector.reciprocal(rec[:], den[:])
    num = mp.tile([128, 25], F32, name=f"num_{rep}", tag="num", bufs=1)
    nc.vector.tensor_mul(num[:], e3a[:, :, 0], v3a[:, :, 0])
    nc.vector.tensor_mul(score_sb[:, 0:25], num[:], rec[:])
    nc.sync.dma_start(
        score.ap()[0:3072].rearrange("(n p) -> p n", p=128),
        score_sb[:, 0:24])
    nc.sync.dma_start(
        score.ap()[3072:3136].rearrange("(n p) -> p n", p=64),
        score_sb[0:64, 24:25])


# ------------------------------------------------------------- host side ----
def _bilinear_1d(n_in, n_out):
    """Half-pixel bilinear (jax.image.resize 'bilinear') 1D matrix
    [n_out, n_in]."""
    u = np.zeros((n_out, n_in), np.float64)
    scale = n_in / n_out
    for o in range(n_out):
        src = (o + 0.5) * scale - 0.5
        i0 = int(np.floor(src))
        f = src - i0
        i0c = min(max(i0, 0), n_in - 1)
        i1c = min(max(i0 + 1, 0), n_in - 1)
        u[o, i0c] += 1.0 - f
        u[o, i1c] += f
    return u


def prep_shared(W, b, C):
    W = np.asarray(W, np.float32)
    b = np.asarray(b, np.float32)
    C = np.asarray(C, np.float32)
    wt_full = W[:, :CO].T / 9.0 * S_W8                     # (1792, 1792)
    wtp = np.ascontiguousarray(
        wt_full[:768].reshape(6, 128, CO)).astype(ml_dtypes.float8_e4m3)
    wt3 = np.ascontiguousarray(
        wt_full[768:].reshape(8, 128, CO)).astype(ml_dtypes.float8_e4m3)
    wco = np.ascontiguousarray(np.concatenate(
        [W[:, CO:CO + 2].T, b[None, :]]) * S_PHI).astype(
        ml_dtypes.float8_e4m3)
    # U: 4x bilinear 14->56, as [partition 128, plane 2, pos 3136] fp8.
    # source pos t=(iy,ix): plane h=iy//7, partition p=(iy%7)*14+ix.
    u1 = _bilinear_1d(14, 56)                              # (56, 14)
    U = np.einsum("ai,bj->abij", u1, u1).reshape(HW, 196)  # [out, (iy ix)]
    uc = np.zeros((128, 2, NCEN), np.float32)
    Ut = U.T.reshape(14, 14, HW)                           # [iy, ix, out]
    for iy in range(14):
        for ix in range(14):
            uc[(iy % 7) * 14 + ix, iy // 7, :] = Ut[iy, ix]
    lin = np.linspace(-1.0, 1.0, 56, dtype=np.float32)
    uc[98, 0, :] = np.broadcast_to(lin[None, :], (56, 56)).reshape(HW)
    uc[99, 0, :] = np.broadcast_to(lin[:, None], (56, 56)).reshape(HW)
    uc[100, 0, :] = 1.0
    ucoord = uc.astype(ml_dtypes.float8_e4m3)
    c2f8 = (2.0 * S_C * C).astype(ml_dtypes.float8_e4m3)
    c2 = np.ascontiguousarray(c2f8.reshape(MCH, 128, NCEN))
    # -SS*|C|^2 of the fp8-quantized centroids, split hi (x64) lo (x8)
    cq = c2f8.astype(np.float64) / (2.0 * S_C)
    cn = SS * (cq ** 2).sum(axis=0)                        # SS*|C|^2
    u1b = (-cn / 64.0).astype(ml_dtypes.float8_e4m3)
    r = -cn - 64.0 * u1b.astype(np.float64)
    u2b = (r / 8.0).astype(ml_dtypes.float8_e4m3)
    cbx = np.concatenate([np.asarray(u1b), np.asarray(u2b)]).reshape(
        1, 2 * NCEN)
    stb = np.zeros((1, 256), np.float32)
    stb[0, 0:128] = 64.0
    stb[0, 128:256] = 8.0
    return {"wtp": wtp, "wt3": wt3, "wco": wco, "ucoord": ucoord,
            "c2": c2, "cbx": cbx,
            "stb": stb.astype(ml_dtypes.float8_e4m3)}


def make_in_maps(p1, p2, p3, W, b, C):
    shared = prep_shared(W, b, C)
    maps = []
    for i in range(N_CORES):
        m = dict(shared)
        m["p1"] = (np.asarray(p1)[i].reshape(256, HW)
                   * X_SCALE).astype(ml_dtypes.bfloat16)
        m["p2"] = (np.asarray(p2)[i].reshape(512, 784)
                   * X_SCALE).astype(ml_dtypes.bfloat16)
        m["p3"] = (np.asarray(p3)[i].reshape(1024, 196)
                   * X_SCALE).astype(ml_dtypes.bfloat16)
        maps.append(m)
    return maps


# ------------------------------------------------------------- runner -------
class _Runtime:
    def __init__(self, repeat=1):
        import jax
        from jax.sharding import Mesh, PartitionSpec
        from jax.experimental.shard_map import shard_map
        from concourse import bass2jax

        self.jax = jax
        self.nc = build_program(repeat=repeat)
        nc = self.nc
        bass2jax.install_neuronx_cc_hook()

        partition_name = (nc.partition_id_tensor.name
                          if nc.partition_id_tensor else None)
        in_names, out_names, out_avals, zero_outs = [], [], [], []
        for alloc in nc.m.functions[0].allocations:
            if not isinstance(alloc, mybir.MemoryLocationSet):
                continue
            name = alloc.memorylocations[0].name
            if alloc.kind == "ExternalInput":
                if name != partition_name:
                    in_names.append(name)
            elif alloc.kind == "ExternalOutput":
                shape = tuple(alloc.tensor_shape)
                dtype = mybir.dt.np(alloc.dtype)
                out_names.append(name)
                out_avals.append(jax.core.ShapedArray(shape, dtype))
                zero_outs.append(np.zeros(shape, dtype))
        self.in_names = list(in_names)
        self.out_names = out_names
        self.out_avals = out_avals
        self.zero_outs = zero_outs
        n_params = len(in_names)
        n_outs = len(out_avals)
        all_in_names = in_names + out_names
        if partition_name is not None:
            all_in_names.append(partition_name)

        def _body(*args):
            operands = list(args)
            if partition_name is not None:
                operands.append(bass2jax.partition_id_tensor())
            outs = bass2jax._bass_exec_p.bind(
                *operands,
                out_avals=tuple(out_avals),
                in_names=tuple(all_in_names),
                out_names=tuple(out_names),
                lowering_input_output_aliases=(),
                sim_require_finite=True,
                sim_require_nnan=True,
                nc=nc,
            )
            return tuple(outs)

        devices = jax.devices()[:N_CORES]
        mesh = Mesh(np.asarray(devices), ("core",))
        self.mesh = mesh
        self.pspec = PartitionSpec("core")
        in_specs = (PartitionSpec("core"),) * (n_params + n_outs)
        out_specs = (PartitionSpec("core"),) * n_outs
        self.sharded = jax.jit(
            shard_map(_body, mesh=mesh, in_specs=in_specs,
                      out_specs=out_specs, check_rep=False),
            donate_argnums=tuple(range(n_params, n_params + n_outs)),
            keep_unused=True,
        )

    def concat_inputs(self, in_maps):
        return [np.concatenate([np.asarray(in_maps[c][nm])
                                for c in range(N_CORES)], axis=0)
                for nm in self.in_names]

    def zeros(self):
        return [np.zeros((N_CORES * z.shape[0], *z.shape[1:]), z.dtype)
                for z in self.zero_outs]

    def device_put_sharded(self, arrays):
        from jax.sharding import NamedSharding
        sh = NamedSharding(self.mesh, self.pspec)
        return [self.jax.device_put(a, sh) for a in arrays]

    def run(self, in_maps):
        outs = self.sharded(*self.concat_inputs(in_maps), *self.zeros())
        res = {}
        for i, nm in enumerate(self.out_names):
            a = np.asarray(outs[i])
            res[nm] = a.reshape(N_CORES, *self.out_avals[i].shape)
        return res


_RT = {}


def _runtime(repeat=1):
    if repeat not in _RT:
        _RT[repeat] = _Runtime(repeat=repeat)
    return _RT[repeat]


def kernel(p1, p2, p3, W, b, C):
    rt = _runtime()
    in_maps = make_in_maps(p1, p2, p3, W, b, C)
    res = rt.run(in_maps)
    return np.ascontiguousarray(
        res["score"].reshape(N_CORES, 1, 56, 56).astype(np.float32))
